# revision 8
# baseline (speedup 1.0000x reference)
"""AttentionBlock kernel for Trainium2 (8 NeuronCores, SPMD).

Problem (hardcoded shapes): x [4, 256, 64, 64] f32, wq/wk [32, 256], bq/bk [32],
wv [256, 256], bv [256], gamma [1].

    xf = x.reshape(B, C, N);  N = 4096
    q = wq @ xf + bq        [B, 32, N]
    k = wk @ xf + bk        [B, 32, N]
    v = wv @ xf + bv        [B, 256, N]
    energy[b,i,j] = sum_c q[b,c,i] k[b,c,j]
    attn = softmax(energy, axis=-1)
    out[b,c,i] = sum_j v[b,c,j] attn[b,i,j]
    result = gamma * out + x

Sharding: 8 cores = 4 batch samples x 2 query-position halves. The host feeds
each core a column-permuted copy of its sample (its query half rotated to the
front) so the SPMD program is identical on every core: queries = cols 0..2047,
keys/values = all 4096 cols (attention sums over j, so key order is irrelevant).

Per-core program:
  - warmup matmul burst at t=0 so the PE HAM clock-gate reaches 8/8 early.
  - k [32, 4096], q [32, 2048] via PE matmuls; both replicated to partitions
    32..63 so energy matmuls can 2-way row-pack (tile_position) into the two
    banks of a [128, 1024] PSUM tile.
  - V^T [4096, 256] via PE (lhsT = x tiles), +bv via a K=1 ones-row matmul,
    scaled by gamma during the PSUM->SBUF copy. vT row stride padded to 264
    (16B-aligned); col 256 is all-ones so the AV matmul also produces softmax
    row-sums; cols 257..263 are zero.
  - Energy E^T[j, i] pairs: two row-packed matmuls fill [128, 1024] PSUM (two
    j-tiles x 512 queries), one exp on ScalarE per pair (softmax without
    max-subtraction: |E| <~ 5 for this data scale). AV with A^T stationary
    (psum [i, 264]), reciprocal of col 256, scale, PE-transpose to [c, i],
    + x residual, DMA out. Emission is software-pipelined one super ahead so
    exp(s+1) overlaps AV(s).
"""

import numpy as np

import concourse.bass as bass
import concourse.mybir as mybir
import concourse.tile as tile
from concourse import bacc
from concourse.bass_utils import run_bass_kernel_spmd
from concourse.masks import make_identity

F32 = mybir.dt.float32
BF16 = mybir.dt.bfloat16
AF = mybir.ActivationFunctionType

B, C, CO, N = 4, 256, 32, 4096
NQ = N // 2          # queries per core
P = 128
SUP = 512            # query super-block
NSUP = NQ // SUP     # 4
NSUB = SUP // P      # 4
JT = N // P          # 32 key tiles
VTW = 264            # V^T row stride: 256 data + ones col + 16B-align pad


def _emit(nc: bass.Bass, tc: tile.TileContext, io: dict):
    x_d, wq_d, bq_d, wk_d, bk_d, wv_d, bv_d, g_d, out_d = (
        io["x"], io["wq"], io["bq"], io["wk"], io["bk"], io["wv"], io["bv"],
        io["gamma"], io["out"])

    with (
        tc.tile_pool(name="x32p", bufs=2) as x32p,
        tc.tile_pool(name="xbfp", bufs=2) as xbfp,
        tc.tile_pool(name="kqp", bufs=1) as kqp,
        tc.tile_pool(name="vtp", bufs=1) as vtp,
        tc.tile_pool(name="wp", bufs=1) as wp,
        tc.tile_pool(name="wstage", bufs=2) as wstage,
        tc.tile_pool(name="atp", bufs=40) as atp,
        tc.tile_pool(name="otp", bufs=3) as otp,
        tc.tile_pool(name="recp", bufs=3) as recp,
        tc.tile_pool(name="resp", bufs=2) as resp,
        tc.tile_pool(name="pe", bufs=2, space="PSUM") as pe_pool,
        tc.tile_pool(name="po", bufs=2, space="PSUM") as po_pool,
        tc.tile_pool(name="pt", bufs=2, space="PSUM") as pt_pool,
    ):
        # ---------- warmup: dense PE burst to flip HAM to 8/8 ----------
        ident = wp.tile([P, P], BF16, tag="ident", name="ident")
        make_identity(nc, ident[:])
        wu = wp.tile([P, SUP], BF16, tag="wu", name="wu")
        nc.gpsimd.memset(wu[:], 0.25)
        for w in range(8):
            wups = pe_pool.tile([P, SUP], F32, tag="pe", name=f"wups_{w}")
            nc.tensor.matmul(wups[:], ident[:], wu[:], start=True, stop=True)

        # ---------- small constants (DMA'd first: tiny, unblocks PE work) ----
        ones_bf = wp.tile([1, P], BF16, tag="ones_bf", name="ones_bf")
        ones_f32 = wp.tile([1, P], F32, tag="ones_f32", name="ones_f32")
        nc.gpsimd.memset(ones_bf[:], 1.0)
        nc.gpsimd.memset(ones_f32[:], 1.0)

        bq_sb = wp.tile([CO, 1], F32, tag="bq", name="bq_sb")
        bk_sb = wp.tile([CO, 1], F32, tag="bk", name="bk_sb")
        nc.sync.dma_start(bq_sb[:], bq_d[:, None])
        nc.sync.dma_start(bk_sb[:], bk_d[:, None])
        bv32 = wp.tile([1, C], F32, tag="bv32", name="bv32")
        bvbf = wp.tile([1, C], BF16, tag="bvbf", name="bvbf")
        nc.sync.dma_start(bv32[:], bv_d[None, :])
        nc.vector.tensor_copy(bvbf[:], bv32[:])
        g32 = wp.tile([1, 1], F32, tag="g32", name="g32")
        nc.sync.dma_start(g32[:], g_d[None, :])

        # gamma broadcast to [128, 1] via ones matmul
        pg = pe_pool.tile([P, 1], F32, tag="pe", name="pg")
        nc.tensor.matmul(pg[:], ones_f32[:], g32[:], start=True, stop=True)
        gamma_b = wp.tile([P, 1], F32, tag="gamma_b", name="gamma_b")
        nc.vector.tensor_copy(gamma_b[:], pg[:])

        # ---------- weight transposes (wqT/wkT [256, 32] as [128, 2*32]; wvT) ----
        wqT = wp.tile([P, 2 * CO], BF16, tag="wqT", name="wqT")
        wkT = wp.tile([P, 2 * CO], BF16, tag="wkT", name="wkT")
        wvT = wp.tile([P, 2 * C], BF16, tag="wvT", name="wvT")

        for nm, wd, wT in (("q", wq_d, wqT), ("k", wk_d, wkT)):
            w32 = wstage.tile([CO, C], F32, tag="wstage32", name=f"w32_{nm}")
            wbf = wstage.tile([CO, C], BF16, tag="wstagebf", name=f"wbf_{nm}")
            nc.sync.dma_start(w32[:], wd[:, :])
            nc.vector.tensor_copy(wbf[:], w32[:])
            for ct in (0, 1):
                ps = pe_pool.tile([P, CO], BF16, tag="pe", name=f"psT_{nm}{ct}")
                nc.tensor.transpose(ps[:], wbf[:, ct * P:(ct + 1) * P],
                                    ident[:CO, :CO])
                nc.vector.tensor_copy(wT[:, ct * CO:(ct + 1) * CO], ps[:])

        for r in (0, 1):
            wv32 = wstage.tile([P, C], F32, tag="wstage32", name=f"wv32_{r}")
            wvbf = wstage.tile([P, C], BF16, tag="wstagebf", name=f"wvbf_{r}")
            nc.sync.dma_start(wv32[:], wv_d[r * P:(r + 1) * P, :])
            nc.vector.tensor_copy(wvbf[:], wv32[:])
            for ct in (0, 1):
                ps = pe_pool.tile([P, P], BF16, tag="pe", name=f"psT_v{r}{ct}")
                nc.tensor.transpose(ps[:], wvbf[:, ct * P:(ct + 1) * P], ident[:])
                nc.vector.tensor_copy(wvT[:, ct * C + r * P:ct * C + (r + 1) * P],
                                      ps[:])

        # ---------- load x and cast to bf16 (casts on GPSIMD: DVE is busy) ----
        x32 = [x32p.tile([P, N], F32, tag="x32", name=f"x32_{t}") for t in (0, 1)]
        xbf = [xbfp.tile([P, N], BF16, tag="xbf", name=f"xbf_{t}") for t in (0, 1)]
        CH = 1024
        for t in (0, 1):
            for c0 in range(0, N, CH):
                nc.sync.dma_start(x32[t][:, c0:c0 + CH],
                                  x_d[t * P:(t + 1) * P, c0:c0 + CH])
                nc.gpsimd.tensor_copy(xbf[t][:, c0:c0 + CH], x32[t][:, c0:c0 + CH])

        # ---------- projections (k/q written to partitions 0..31, then
        # replicated to 32..63 for 2-way row-packed energy matmuls) ----------
        k_rep = kqp.tile([2 * CO, N], BF16, tag="k_rep", name="k_rep")
        q_rep = kqp.tile([2 * CO, NQ], BF16, tag="q_rep", name="q_rep")
        for c0 in range(0, N, SUP):
            pk = pe_pool.tile([CO, SUP], F32, tag="pe", name=f"pk_{c0}")
            nc.tensor.matmul(pk[:], wkT[:, 0:CO], xbf[0][:, c0:c0 + SUP],
                             start=True, stop=False)
            nc.tensor.matmul(pk[:], wkT[:, CO:2 * CO], xbf[1][:, c0:c0 + SUP],
                             start=False, stop=True)
            nc.scalar.activation(k_rep[0:CO, c0:c0 + SUP], pk[:], AF.Identity,
                                 bias=bk_sb[:])
        for c0 in range(0, NQ, SUP):
            pq = pe_pool.tile([CO, SUP], F32, tag="pe", name=f"pq_{c0}")
            nc.tensor.matmul(pq[:], wqT[:, 0:CO], xbf[0][:, c0:c0 + SUP],
                             start=True, stop=False)
            nc.tensor.matmul(pq[:], wqT[:, CO:2 * CO], xbf[1][:, c0:c0 + SUP],
                             start=False, stop=True)
            nc.scalar.activation(q_rep[0:CO, c0:c0 + SUP], pq[:], AF.Identity,
                                 bias=bq_sb[:])
        nc.sync.dma_start(k_rep[CO:2 * CO, :], k_rep[0:CO, :])
        nc.sync.dma_start(q_rep[CO:2 * CO, :], q_rep[0:CO, :])

        # ---------- V^T (gamma folded in via ACT scale) + ones/pad columns ----
        vT = vtp.tile([P, JT * VTW], BF16, tag="vT", name="vT")
        nc.gpsimd.memset(vT[:], 0.0)
        for jt in range(JT):
            nc.gpsimd.memset(vT[:, jt * VTW + C:jt * VTW + C + 1], 1.0)

        def build_vt(jt):
            pv = pe_pool.tile([P, C], F32, tag="pe", name=f"pv_{jt}")
            nc.tensor.matmul(pv[:], xbf[0][:, jt * P:(jt + 1) * P], wvT[:, 0:C],
                             start=True, stop=False)
            nc.tensor.matmul(pv[:], xbf[1][:, jt * P:(jt + 1) * P], wvT[:, C:2 * C],
                             start=False, stop=False)
            nc.tensor.matmul(pv[:], ones_bf[:], bvbf[:], start=False, stop=True)
            nc.vector.tensor_scalar_mul(vT[:, jt * VTW:jt * VTW + C], pv[:],
                                        gamma_b[:])

        # ---------- attention super-blocks, software-pipelined ----------
        at_tiles = {}   # s -> list of 16 [128, 1024] tiles (two j-tiles each)

        def energy_exp(s):
            i0 = s * SUP
            tiles = []
            for jp in range(JT // 2):
                pe_t = pe_pool.tile([P, 2 * SUP], F32, tag="pe", name=f"pe_{s}_{jp}")
                for h in (0, 1):
                    jt = 2 * jp + h
                    nc.tensor.matmul(pe_t[:, h * SUP:(h + 1) * SUP],
                                     k_rep[h * CO:(h + 1) * CO,
                                           jt * P:(jt + 1) * P],
                                     q_rep[h * CO:(h + 1) * CO, i0:i0 + SUP],
                                     start=True, stop=True,
                                     tile_position=(h * CO, 0))
                at = atp.tile([P, 2 * SUP], BF16, tag="AT", name=f"at_{s}_{jp}")
                nc.scalar.activation(at[:], pe_t[:], AF.Exp)
                tiles.append(at)
            at_tiles[s] = tiles

        def av_epilogue(s):
            i0 = s * SUP
            tiles = at_tiles.pop(s)
            res = [resp.tile([P, SUP], F32, tag=f"res{ct}", name=f"res_{s}_{ct}")
                   for ct in (0, 1)]
            for sub in range(NSUB):
                po = po_pool.tile([P, VTW], F32, tag="po", name=f"po_{s}_{sub}")
                for jt in range(JT):
                    lhs = tiles[jt // 2][:, (jt % 2) * SUP + sub * P:
                                         (jt % 2) * SUP + (sub + 1) * P]
                    nc.tensor.matmul(po[:], lhs, vT[:, jt * VTW:(jt + 1) * VTW],
                                     start=(jt == 0), stop=(jt == JT - 1))
                rec = recp.tile([P, 1], F32, tag="rec", name=f"rec_{s}_{sub}")
                nc.vector.reciprocal(rec[:], po[:, C:C + 1])
                oT = otp.tile([P, C], BF16, tag="oT", name=f"oT_{s}_{sub}")
                nc.vector.tensor_scalar_mul(oT[:], po[:, 0:C], rec[:])
                for ct in (0, 1):
                    pt = pt_pool.tile([P, P], BF16, tag="pt", name=f"pt_{s}_{sub}_{ct}")
                    nc.tensor.transpose(pt[:], oT[:, ct * P:(ct + 1) * P], ident[:])
                    nc.vector.tensor_add(res[ct][:, sub * P:(sub + 1) * P], pt[:],
                                         x32[ct][:, i0 + sub * P:i0 + (sub + 1) * P])
            for ct in (0, 1):
                nc.sync.dma_start(out_d[ct * P:(ct + 1) * P, i0:i0 + SUP],
                                  res[ct][:])

        # energy/exp first so ScalarE (the pacer) starts ASAP; vT is built on
        # PE while exp streams; AV(s) overlaps exp(s+1).
        energy_exp(0)
        energy_exp(1)
        for jt in range(JT):
            build_vt(jt)
        av_epilogue(0)
        energy_exp(2)
        av_epilogue(1)
        energy_exp(3)
        av_epilogue(2)
        av_epilogue(3)


_CACHE = {}


def _build():
    if "nc" in _CACHE:
        return _CACHE["nc"]
    nc = bacc.Bacc("TRN2", target_bir_lowering=False, debug=False, num_devices=8)
    io = {
        "x": nc.dram_tensor("x", [C, N], F32, kind="ExternalInput").ap(),
        "wq": nc.dram_tensor("wq", [CO, C], F32, kind="ExternalInput").ap(),
        "bq": nc.dram_tensor("bq", [CO], F32, kind="ExternalInput").ap(),
        "wk": nc.dram_tensor("wk", [CO, C], F32, kind="ExternalInput").ap(),
        "bk": nc.dram_tensor("bk", [CO], F32, kind="ExternalInput").ap(),
        "wv": nc.dram_tensor("wv", [C, C], F32, kind="ExternalInput").ap(),
        "bv": nc.dram_tensor("bv", [C], F32, kind="ExternalInput").ap(),
        "gamma": nc.dram_tensor("gamma", [1], F32, kind="ExternalInput").ap(),
        "out": nc.dram_tensor("out", [C, NQ], F32, kind="ExternalOutput").ap(),
    }
    with tile.TileContext(nc) as tc:
        _emit(nc, tc, io)
    nc.compile()
    _CACHE["nc"] = nc
    return nc


def run_spmd(inputs: dict, **kw):
    nc = _build()
    x = np.ascontiguousarray(np.asarray(inputs["x"], dtype=np.float32))
    shared = {
        name: np.ascontiguousarray(np.asarray(inputs[name], dtype=np.float32))
        for name in ("wq", "bq", "wk", "bk", "wv", "bv", "gamma")
    }
    in_maps = []
    for core in range(8):
        b, half = core // 2, core % 2
        xf = x[b].reshape(C, N)
        if half:
            xf = np.concatenate([xf[:, NQ:], xf[:, :NQ]], axis=1)
        in_maps.append({"x": np.ascontiguousarray(xf), **shared})
    r = run_bass_kernel_spmd(nc, in_maps, core_ids=list(range(8)), **kw)
    out = np.empty((B, C, N), dtype=np.float32)
    for core in range(8):
        b, half = core // 2, core % 2
        out[b][:, half * NQ:(half + 1) * NQ] = r.results[core]["out"]
    return out.reshape(B, C, 64, 64), r


def kernel(**inputs) -> np.ndarray:
    out, _ = run_spmd(inputs)
    return out


# revision 11
# speedup vs baseline: 1.6716x; 1.6716x over previous
"""AttentionBlock kernel for Trainium2 (8 NeuronCores, SPMD).

Problem (hardcoded shapes): x [4, 256, 64, 64] f32, wq/wk [32, 256], bq/bk [32],
wv [256, 256], bv [256], gamma [1].

    xf = x.reshape(B, C, N);  N = 4096
    q = wq @ xf + bq        [B, 32, N]
    k = wk @ xf + bk        [B, 32, N]
    v = wv @ xf + bv        [B, 256, N]
    energy[b,i,j] = sum_c q[b,c,i] k[b,c,j]
    attn = softmax(energy, axis=-1)
    out[b,c,i] = sum_j v[b,c,j] attn[b,i,j]
    result = gamma * out + x

Sharding: 8 cores = 4 batch samples x 2 query-position halves. The host feeds
each core a column-permuted copy of its sample (its query half rotated to the
front) so the SPMD program is identical on every core: queries = cols 0..2047,
keys/values = all 4096 cols (attention sums over j, so key order is irrelevant).

Per-core program:
  - warmup matmul burst at t=0 so the PE HAM clock-gate reaches 8/8 early.
  - k [32, 4096], q [32, 2048] via PE matmuls; both replicated to partitions
    32..63 so energy matmuls can 2-way row-pack (tile_position) into the two
    banks of a [128, 1024] PSUM tile.
  - V^T [4096, 256] via PE (lhsT = x tiles), +bv via a K=1 ones-row matmul,
    scaled by gamma during the PSUM->SBUF copy. vT row stride padded to 264
    (16B-aligned); col 256 is all-ones so the AV matmul also produces softmax
    row-sums; cols 257..263 are zero.
  - Energy E^T[j, i] pairs: two row-packed matmuls fill [128, 1024] PSUM (two
    j-tiles x 512 queries), one exp on ScalarE per pair (softmax without
    max-subtraction: |E| <~ 5 for this data scale). AV with A^T stationary
    (psum [i, 264]), reciprocal of col 256, scale, PE-transpose to [c, i],
    + x residual, DMA out. Emission is software-pipelined one super ahead so
    exp(s+1) overlaps AV(s).
"""

import numpy as np

import concourse.bass as bass
import concourse.mybir as mybir
import concourse.tile as tile
from concourse import bacc
from concourse.bass_utils import run_bass_kernel_spmd
from concourse.masks import make_identity

F32 = mybir.dt.float32
BF16 = mybir.dt.bfloat16
AF = mybir.ActivationFunctionType

B, C, CO, N = 4, 256, 32, 4096
NQ = N // 2          # queries per core
P = 128
SUP = 512            # query super-block
NSUP = NQ // SUP     # 4
NSUB = SUP // P      # 4
JT = N // P          # 32 key tiles
VTW = 264            # V^T row stride: 256 data + ones col + 16B-align pad


def _emit(nc: bass.Bass, tc: tile.TileContext, io: dict):
    x_d, wq_d, bq_d, wk_d, bk_d, wv_d, bv_d, g_d, out_d = (
        io["x"], io["wq"], io["bq"], io["wk"], io["bk"], io["wv"], io["bv"],
        io["gamma"], io["out"])

    with (
        tc.tile_pool(name="x32p", bufs=2) as x32p,
        tc.tile_pool(name="xbfp", bufs=2) as xbfp,
        tc.tile_pool(name="kqp", bufs=1) as kqp,
        tc.tile_pool(name="vtp", bufs=1) as vtp,
        tc.tile_pool(name="wp", bufs=1) as wp,
        tc.tile_pool(name="wstage", bufs=2) as wstage,
        tc.tile_pool(name="atp", bufs=40) as atp,
        tc.tile_pool(name="otp", bufs=3) as otp,
        tc.tile_pool(name="recp", bufs=3) as recp,
        tc.tile_pool(name="resp", bufs=2) as resp,
        tc.tile_pool(name="pe", bufs=2, space="PSUM") as pe_pool,
        tc.tile_pool(name="po", bufs=2, space="PSUM") as po_pool,
        tc.tile_pool(name="pt", bufs=2, space="PSUM") as pt_pool,
    ):
        # ---------- warmup: dense PE burst to flip HAM to 8/8 ----------
        ident = wp.tile([P, P], BF16, tag="ident", name="ident")
        make_identity(nc, ident[:])
        wu = wp.tile([P, SUP], BF16, tag="wu", name="wu")
        nc.gpsimd.memset(wu[:], 0.25)
        for w in range(8):
            wups = pe_pool.tile([P, SUP], F32, tag="pe", name=f"wups_{w}")
            nc.tensor.matmul(wups[:], ident[:], wu[:], start=True, stop=True)

        # ---------- small constants (DMA'd first: tiny, unblocks PE work) ----
        ones_bf = wp.tile([1, P], BF16, tag="ones_bf", name="ones_bf")
        ones_f32 = wp.tile([1, P], F32, tag="ones_f32", name="ones_f32")
        nc.gpsimd.memset(ones_bf[:], 1.0)
        nc.gpsimd.memset(ones_f32[:], 1.0)

        bq_sb = wp.tile([CO, 1], F32, tag="bq", name="bq_sb")
        bk_sb = wp.tile([CO, 1], F32, tag="bk", name="bk_sb")
        nc.sync.dma_start(bq_sb[:], bq_d[:, None])
        nc.sync.dma_start(bk_sb[:], bk_d[:, None])
        bv32 = wp.tile([1, C], F32, tag="bv32", name="bv32")
        bvbf = wp.tile([1, C], BF16, tag="bvbf", name="bvbf")
        nc.sync.dma_start(bv32[:], bv_d[None, :])
        nc.vector.tensor_copy(bvbf[:], bv32[:])
        g32 = wp.tile([1, 1], F32, tag="g32", name="g32")
        nc.sync.dma_start(g32[:], g_d[None, :])

        # gamma broadcast to [128, 1] via ones matmul
        pg = pe_pool.tile([P, 1], F32, tag="pe", name="pg")
        nc.tensor.matmul(pg[:], ones_f32[:], g32[:], start=True, stop=True)
        gamma_b = wp.tile([P, 1], F32, tag="gamma_b", name="gamma_b")
        nc.vector.tensor_copy(gamma_b[:], pg[:])

        # ---------- weight transposes (wqT/wkT [256, 32] as [128, 2*32]; wvT) ----
        wqT = wp.tile([P, 2 * CO], BF16, tag="wqT", name="wqT")
        wkT = wp.tile([P, 2 * CO], BF16, tag="wkT", name="wkT")
        wvT = wp.tile([P, 2 * C], BF16, tag="wvT", name="wvT")

        for nm, wd, wT in (("q", wq_d, wqT), ("k", wk_d, wkT)):
            w32 = wstage.tile([CO, C], F32, tag="wstage32", name=f"w32_{nm}")
            wbf = wstage.tile([CO, C], BF16, tag="wstagebf", name=f"wbf_{nm}")
            nc.sync.dma_start(w32[:], wd[:, :])
            nc.vector.tensor_copy(wbf[:], w32[:])
            for ct in (0, 1):
                ps = pe_pool.tile([P, CO], BF16, tag="pe", name=f"psT_{nm}{ct}")
                nc.tensor.transpose(ps[:], wbf[:, ct * P:(ct + 1) * P],
                                    ident[:CO, :CO])
                nc.vector.tensor_copy(wT[:, ct * CO:(ct + 1) * CO], ps[:])

        for r in (0, 1):
            wv32 = wstage.tile([P, C], F32, tag="wstage32", name=f"wv32_{r}")
            wvbf = wstage.tile([P, C], BF16, tag="wstagebf", name=f"wvbf_{r}")
            nc.sync.dma_start(wv32[:], wv_d[r * P:(r + 1) * P, :])
            nc.vector.tensor_copy(wvbf[:], wv32[:])
            for ct in (0, 1):
                ps = pe_pool.tile([P, P], BF16, tag="pe", name=f"psT_v{r}{ct}")
                nc.tensor.transpose(ps[:], wvbf[:, ct * P:(ct + 1) * P], ident[:])
                nc.vector.tensor_copy(wvT[:, ct * C + r * P:ct * C + (r + 1) * P],
                                      ps[:])

        # ---------- load x and cast to bf16 (casts on GPSIMD: DVE is busy) ----
        x32 = [x32p.tile([P, N], F32, tag="x32", name=f"x32_{t}") for t in (0, 1)]
        xbf = [xbfp.tile([P, N], BF16, tag="xbf", name=f"xbf_{t}") for t in (0, 1)]
        CH = 1024
        for t in (0, 1):
            for c0 in range(0, N, CH):
                nc.sync.dma_start(x32[t][:, c0:c0 + CH],
                                  x_d[t * P:(t + 1) * P, c0:c0 + CH])
                nc.vector.tensor_copy(xbf[t][:, c0:c0 + CH], x32[t][:, c0:c0 + CH])

        # ---------- projections (k/q written to partitions 0..31, then
        # replicated to 32..63 for 2-way row-packed energy matmuls) ----------
        k_rep = kqp.tile([2 * CO, N], BF16, tag="k_rep", name="k_rep")
        q_rep = kqp.tile([2 * CO, NQ], BF16, tag="q_rep", name="q_rep")
        for c0 in range(0, N, SUP):
            pk = pe_pool.tile([CO, SUP], F32, tag="pe", name=f"pk_{c0}")
            nc.tensor.matmul(pk[:], wkT[:, 0:CO], xbf[0][:, c0:c0 + SUP],
                             start=True, stop=False)
            nc.tensor.matmul(pk[:], wkT[:, CO:2 * CO], xbf[1][:, c0:c0 + SUP],
                             start=False, stop=True)
            nc.scalar.activation(k_rep[0:CO, c0:c0 + SUP], pk[:], AF.Identity,
                                 bias=bk_sb[:])
            nc.sync.dma_start(k_rep[CO:2 * CO, c0:c0 + SUP],
                              k_rep[0:CO, c0:c0 + SUP])
        for c0 in range(0, NQ, SUP):
            pq = pe_pool.tile([CO, SUP], F32, tag="pe", name=f"pq_{c0}")
            nc.tensor.matmul(pq[:], wqT[:, 0:CO], xbf[0][:, c0:c0 + SUP],
                             start=True, stop=False)
            nc.tensor.matmul(pq[:], wqT[:, CO:2 * CO], xbf[1][:, c0:c0 + SUP],
                             start=False, stop=True)
            nc.scalar.activation(q_rep[0:CO, c0:c0 + SUP], pq[:], AF.Identity,
                                 bias=bq_sb[:])
            nc.sync.dma_start(q_rep[CO:2 * CO, c0:c0 + SUP],
                              q_rep[0:CO, c0:c0 + SUP])

        # ---------- V^T (gamma folded in) + ones/pad columns ----------
        vT = vtp.tile([P, JT * VTW], BF16, tag="vT", name="vT")
        for jt in range(JT):
            nc.gpsimd.memset(vT[:, jt * VTW + C:jt * VTW + C + 1], 1.0)
            nc.gpsimd.memset(vT[:, jt * VTW + C + 1:(jt + 1) * VTW], 0.0)

        def build_vt(jt):
            pv = pt_pool.tile([P, C], F32, tag="pt", name=f"pv_{jt}")
            nc.tensor.matmul(pv[:], xbf[0][:, jt * P:(jt + 1) * P], wvT[:, 0:C],
                             start=True, stop=False)
            nc.tensor.matmul(pv[:], xbf[1][:, jt * P:(jt + 1) * P], wvT[:, C:2 * C],
                             start=False, stop=False)
            nc.tensor.matmul(pv[:], ones_bf[:], bvbf[:], start=False, stop=True)
            nc.vector.tensor_scalar_mul(vT[:, jt * VTW:jt * VTW + C], pv[:],
                                        gamma_b[:])

        # ---------- attention super-blocks, software-pipelined ----------
        at_tiles = {}   # s -> list of 16 [128, 1024] tiles (two j-tiles each)

        def energy_exp(s):
            i0 = s * SUP
            tiles = []
            for jp in range(JT // 2):
                pe_t = pe_pool.tile([P, 2 * SUP], F32, tag="pe", name=f"pe_{s}_{jp}")
                for h in (0, 1):
                    jt = 2 * jp + h
                    nc.tensor.matmul(pe_t[:, h * SUP:(h + 1) * SUP],
                                     k_rep[h * CO:(h + 1) * CO,
                                           jt * P:(jt + 1) * P],
                                     q_rep[h * CO:(h + 1) * CO, i0:i0 + SUP],
                                     start=True, stop=True,
                                     tile_position=(h * CO, 0))
                at = atp.tile([P, 2 * SUP], BF16, tag="AT", name=f"at_{s}_{jp}")
                nc.scalar.activation(at[:], pe_t[:], AF.Exp)
                tiles.append(at)
            at_tiles[s] = tiles

        def av_epilogue(s):
            i0 = s * SUP
            tiles = at_tiles.pop(s)
            res = [resp.tile([P, SUP], F32, tag=f"res{ct}", name=f"res_{s}_{ct}")
                   for ct in (0, 1)]
            for sub in range(NSUB):
                po = po_pool.tile([P, VTW], F32, tag="po", name=f"po_{s}_{sub}")
                for jt in range(JT):
                    lhs = tiles[jt // 2][:, (jt % 2) * SUP + sub * P:
                                         (jt % 2) * SUP + (sub + 1) * P]
                    nc.tensor.matmul(po[:], lhs, vT[:, jt * VTW:(jt + 1) * VTW],
                                     start=(jt == 0), stop=(jt == JT - 1))
                rec = recp.tile([P, 1], F32, tag="rec", name=f"rec_{s}_{sub}")
                nc.vector.reciprocal(rec[:], po[:, C:C + 1])
                oT = otp.tile([P, C], BF16, tag="oT", name=f"oT_{s}_{sub}")
                nc.vector.tensor_scalar_mul(oT[:], po[:, 0:C], rec[:])
                for ct in (0, 1):
                    pt = pt_pool.tile([P, P], BF16, tag="pt", name=f"pt_{s}_{sub}_{ct}")
                    nc.tensor.transpose(pt[:], oT[:, ct * P:(ct + 1) * P], ident[:])
                    nc.vector.tensor_add(res[ct][:, sub * P:(sub + 1) * P], pt[:],
                                         x32[ct][:, i0 + sub * P:i0 + (sub + 1) * P])
            for ct in (0, 1):
                nc.sync.dma_start(out_d[ct * P:(ct + 1) * P, i0:i0 + SUP],
                                  res[ct][:])

        # energy/exp first so ScalarE (the pacer) starts ASAP; vT is built on
        # PE while exp streams; AV(s) overlaps exp(s+1).
        energy_exp(0)
        energy_exp(1)
        for jt in range(JT):
            build_vt(jt)
        av_epilogue(0)
        energy_exp(2)
        av_epilogue(1)
        energy_exp(3)
        av_epilogue(2)
        av_epilogue(3)


_CACHE = {}


def _build():
    if "nc" in _CACHE:
        return _CACHE["nc"]
    nc = bacc.Bacc("TRN2", target_bir_lowering=False, debug=False, num_devices=8)
    io = {
        "x": nc.dram_tensor("x", [C, N], F32, kind="ExternalInput").ap(),
        "wq": nc.dram_tensor("wq", [CO, C], F32, kind="ExternalInput").ap(),
        "bq": nc.dram_tensor("bq", [CO], F32, kind="ExternalInput").ap(),
        "wk": nc.dram_tensor("wk", [CO, C], F32, kind="ExternalInput").ap(),
        "bk": nc.dram_tensor("bk", [CO], F32, kind="ExternalInput").ap(),
        "wv": nc.dram_tensor("wv", [C, C], F32, kind="ExternalInput").ap(),
        "bv": nc.dram_tensor("bv", [C], F32, kind="ExternalInput").ap(),
        "gamma": nc.dram_tensor("gamma", [1], F32, kind="ExternalInput").ap(),
        "out": nc.dram_tensor("out", [C, NQ], F32, kind="ExternalOutput").ap(),
    }
    with tile.TileContext(nc) as tc:
        _emit(nc, tc, io)
    nc.compile()
    _CACHE["nc"] = nc
    return nc


def run_spmd(inputs: dict, **kw):
    nc = _build()
    x = np.ascontiguousarray(np.asarray(inputs["x"], dtype=np.float32))
    shared = {
        name: np.ascontiguousarray(np.asarray(inputs[name], dtype=np.float32))
        for name in ("wq", "bq", "wk", "bk", "wv", "bv", "gamma")
    }
    in_maps = []
    for core in range(8):
        b, half = core // 2, core % 2
        xf = x[b].reshape(C, N)
        if half:
            xf = np.concatenate([xf[:, NQ:], xf[:, :NQ]], axis=1)
        in_maps.append({"x": np.ascontiguousarray(xf), **shared})
    r = run_bass_kernel_spmd(nc, in_maps, core_ids=list(range(8)), **kw)
    out = np.empty((B, C, N), dtype=np.float32)
    for core in range(8):
        b, half = core // 2, core % 2
        out[b][:, half * NQ:(half + 1) * NQ] = r.results[core]["out"]
    return out.reshape(B, C, 64, 64), r


def kernel(**inputs) -> np.ndarray:
    out, _ = run_spmd(inputs)
    return out
